# revision 30
# baseline (speedup 1.0000x reference)
"""BD3LM block-diffusion decoder layer on 8 trn2 NeuronCores.

Sharding: core = 2*b + g  (b = batch 0..3, g = head-group 0..1, 8 heads each).
Each core: QKV projections for its batch/head-group, sparse BD3LM attention
(only ~80 of 256 score tiles per head), O-projection against its Wo row-slice.
Host: sums the two group partials per batch and adds the (bv @ Wo + bo)
correction (softmax rows sum to 1, so the v-bias contributes exactly bv @ Wo).

v2 layout/perf notes:
  - projections in f32r (N=512 -> 1 cyc/row); q/k/v stored bf16 so every
    attention matmul runs 1 cyc/row at ANY free size (f32r is 4 cyc/row
    below N=256, which made the 128-wide diagonal tiles 4x slow in v1).
  - scores computed transposed [k_tile=128, q_span] into a [128,1024] PSUM
    tile per (head, half, j); ONE exp per span (ACT per-op overhead is
    ~293ns, so fewer/bigger activations) and one batched exp for the 8
    block-diagonal tiles.
  - head pairs processed together; score matmuls for the two heads use
    tile_position (0,0)/(64,0) so both K=64 matmuls occupy disjoint PE row
    groups and run concurrently.
  - softmax denominators via a ones-column in v (row 64 of ctx); the
    [1,1024] reciprocal uses reciprocal_approx_fast (the exact DVE
    reciprocal is ~6 cyc/elem on the free dim: 105us of the v1 trace).
  - normalize multiplies read ctx and the PE-broadcast recip directly from
    PSUM (no SBUF staging copies).
"""

import numpy as np

import concourse.bass as bass
import concourse.mybir as mybir
import concourse.tile as tile
from concourse import bacc
from concourse.bass_utils import run_bass_kernel_spmd

F32 = mybir.dt.float32
F32R = mybir.dt.float32r
BF16 = mybir.dt.bfloat16
Act = mybir.ActivationFunctionType

B, T, D = 4, 2048, 1024
H, HD = 16, 64
L = T // 2           # 1024, length of each of [xt | x0]
BS = 4               # block size
G = 2                # head groups (cores per batch)
DG = D // G          # 512 channels per group
HG = H // G          # 8 heads per core
P = 128
NT = L // P          # 8 key/query tiles per half
KC = D // P          # 8 contraction chunks
DT4 = DG // P        # 4 output-partition tiles for qT/kT

PROJ_DT = F32R       # x/W stay fp32-exact; f32r streams 1 cyc/row at N=512

REPEAT = 1  # loop whole computation inside the NEFF (timing experiments only)
DBG = False

_CACHE = {}


def _chunks512(a0, a1):
    """Split [a0, a1) at multiples of 512 (PSUM bank boundaries)."""
    out = []
    while a0 < a1:
        b1 = min(a1, (a0 // 512 + 1) * 512)
        out.append((a0, b1))
        a0 = b1
    return out


def _mm(ap, dt):
    return ap.bitcast(dt) if dt != F32 else ap


def _build():
    import concourse.tile_utils as tile_utils

    tile_utils.max_sbuf_usage = 204 * 1024

    nc = bacc.Bacc("TRN2", target_bir_lowering=False, debug=False, num_devices=8)

    xT = nc.dram_tensor("xT", [D, T], F32, kind="ExternalInput").ap()
    wq = nc.dram_tensor("wq", [D, DG], F32, kind="ExternalInput").ap()
    wk = nc.dram_tensor("wk", [D, DG], F32, kind="ExternalInput").ap()
    wv = nc.dram_tensor("wv", [D, DG], F32, kind="ExternalInput").ap()
    wo = nc.dram_tensor("wo", [DG, D], BF16, kind="ExternalInput").ap()
    bqs = nc.dram_tensor("bqs", [DG], F32, kind="ExternalInput").ap()
    bks = nc.dram_tensor("bks", [DG], F32, kind="ExternalInput").ap()
    msk = nc.dram_tensor("msk", [3, P, P], BF16, kind="ExternalInput").ap()
    out = nc.dram_tensor("out", [T, D], F32, kind="ExternalOutput").ap()

    dbg = {}
    if DBG:
        for nm, shp, dt in (
            ("dbg_qT", [P, DT4, T], BF16),
            ("dbg_kT", [P, DT4, T], BF16),
            ("dbg_v", [P, T // P, HG * 2 * HD], BF16),
            ("dbg_ctxT", [P, DT4, T], BF16),
            ("dbg_recip", [16, 1, L], F32),
            ("dbg_rb", [16, P, L], F32),
            ("dbg_at", [P, 1024], BF16),
            ("dbg_atd", [P, 1024], BF16),
            ("dbg_num", [4, HD, L], F32),
            ("dbg_cs", [4, HD, L], BF16),
        ):
            dbg[nm] = nc.dram_tensor(nm, shp, dt, kind="ExternalOutput").ap()

    views = dict(
        dbg=dbg,
        xT_v=xT.rearrange("(kc p) t -> p kc t", p=P),    # [128, 8, 2048]
        wq_v=wq.rearrange("(kc p) m -> p kc m", p=P),    # [128, 8, 512]
        wk_v=wk.rearrange("(kc p) m -> p kc m", p=P),
        wv_v=wv.rearrange("(kc p) m -> p kc m", p=P),
        wo_v=wo.rearrange("(cc p) n -> p cc n", p=P),    # [128, 4, 1024]
        msk=msk,
        out=out,
    )

    with tile.TileContext(nc) as tc:
        with tc.tile_pool(name="persist", bufs=1) as pers:
            st = dict(
                qT_sb=pers.tile([P, DT4, T], BF16, name="qT_sb"),
                kT_sb=pers.tile([P, DT4, T], BF16, name="kT_sb"),
                # per head a [*, 128] stationary block: col 0 = ones (softmax
                # denominator -> ctx row 0), cols 64..127 = v channels (ctx
                # rows 64..127, base-64-aligned for the normalize multiply),
                # cols 1..63 = zeros. One matmul per chunk, one accumulation
                # group, denominator lands on physical partition 0 where
                # reciprocal_approx_fast + gpsimd partition_broadcast work.
                v_sb=pers.tile([P, T // P, HG * 2 * HD], BF16, name="v_sb"),
                bq_sb=pers.tile([P, DT4], F32, name="bq_sb"),
                bk_sb=pers.tile([P, DT4], F32, name="bk_sb"),
            )
            nc.sync.dma_start(st["bq_sb"], bqs.rearrange("(c p) -> p c", p=P))
            nc.sync.dma_start(st["bk_sb"], bks.rearrange("(c p) -> p c", p=P))
            nc.vector.memset(st["v_sb"], 0.0)
            ones_v = st["v_sb"].rearrange("p t (h c) -> p (t h) c", c=2 * HD)[
                :, :, 0:1
            ]
            nc.vector.memset(ones_v, 1.0)

            for _rep in range(REPEAT):
                _phases(nc, tc, st, views)

    nc.compile()
    return nc


def _phases(nc, tc, st, views):
    qT_sb, kT_sb, v_sb = st["qT_sb"], st["kT_sb"], st["v_sb"]
    xT_v, wo_v, msk, out = views["xT_v"], views["wo_v"], views["msk"], views["out"]

    # ---------------- Phase A: QKV projections (one x stream) ----------------
    with (
        tc.tile_pool(name="wpool", bufs=1) as wpool,
        tc.tile_pool(name="xpool", bufs=3) as xpool,
        tc.tile_pool(name="ppsum", bufs=4, space="PSUM") as ppsum,
        tc.tile_pool(name="vpsum", bufs=4, space="PSUM") as vpsum,
    ):
        wq_sb = wpool.tile([P, KC, DG], F32, name="wq_sb")
        wk_sb = wpool.tile([P, KC, DG], F32, name="wk_sb")
        wv_sb = wpool.tile([P, KC, DG], F32, name="wv_sb")
        x_tiles = []
        for s in range(T // 512):
            x_sb = xpool.tile([P, KC, 512], F32, tag="x", name=f"x{s}")
            if s == 0:
                # per-kc split: the (d4=0, kc=0) matmul unblocks after 1/8
                # of the slab instead of the whole 2MB transfer
                for kc in range(KC):
                    nc.sync.dma_start(
                        _mm(x_sb[:, kc], PROJ_DT),
                        _mm(xT_v[:, kc, 0:512], PROJ_DT),
                    )
            elif s == 1:
                nc.sync.dma_start(
                    _mm(x_sb, PROJ_DT),
                    _mm(xT_v[:, :, 512 * s : 512 * (s + 1)], PROJ_DT),
                )
            x_tiles.append(x_sb)
        # wq split per column-tile: the d4=0 matmuls only wait on 0.5MB of wq
        for d4 in range(DT4):
            nc.sync.dma_start(
                _mm(wq_sb[:, :, P * d4 : P * (d4 + 1)], PROJ_DT),
                _mm(views["wq_v"][:, :, P * d4 : P * (d4 + 1)], PROJ_DT),
            )
        nc.sync.dma_start(_mm(wk_sb, PROJ_DT), _mm(views["wk_v"], PROJ_DT))
        nc.sync.dma_start(_mm(wv_sb, PROJ_DT), _mm(views["wv_v"], PROJ_DT))
        for s in range(T // 512):
            x_sb = x_tiles[s]
            if s >= 2:
                nc.sync.dma_start(
                    _mm(x_sb, PROJ_DT),
                    _mm(xT_v[:, :, 512 * s : 512 * (s + 1)], PROJ_DT),
                )
            for w_sb, b_key, dst, scale in (
                (wq_sb, "bq_sb", qT_sb, HD ** -0.5),
                (wk_sb, "bk_sb", kT_sb, 1.0),
            ):
                for d4 in range(DT4):
                    ps = ppsum.tile([P, 512], F32, tag="pp", name=f"pp{s}_{d4}")
                    for kc in range(KC):
                        nc.tensor.matmul(
                            ps,
                            _mm(w_sb[:, kc, P * d4 : P * (d4 + 1)], PROJ_DT),
                            _mm(x_sb[:, kc, :], PROJ_DT),
                            start=(kc == 0),
                            stop=(kc == KC - 1),
                        )
                    nc.scalar.activation(
                        dst[:, d4, 512 * s : 512 * (s + 1)],
                        ps,
                        Act.Identity,
                        bias=st[b_key][:, d4 : d4 + 1],
                        scale=scale,
                    )
            for t2 in range(4):
                tt = 4 * s + t2
                ps = vpsum.tile([P, DG], F32, tag="ppv", name=f"ppv{tt}")
                for kc in range(KC):
                    nc.tensor.matmul(
                        ps,
                        _mm(x_sb[:, kc, P * t2 : P * (t2 + 1)], PROJ_DT),
                        _mm(wv_sb[:, kc, :], PROJ_DT),
                        start=(kc == 0),
                        stop=(kc == KC - 1),
                    )
                nc.vector.tensor_copy(
                    v_sb[:, tt].rearrange("p (h c) -> p h c", c=2 * HD)[
                        :, :, HD : 2 * HD
                    ],
                    ps.rearrange("p (h c) -> p h c", c=HD),
                )

    # ---------------- Phase B: sparse attention ----------------
    from contextlib import ExitStack as _ES

    with (
        tc.tile_pool(name="apool", bufs=1) as apool,
        tc.tile_pool(name="tmppool", bufs=2) as tmppool,
    ):
        _es = _ES()
        atpool = _es.enter_context(tc.tile_pool(name="atpool", bufs=8))
        spsum = _es.enter_context(tc.tile_pool(name="spsum", bufs=2, space="PSUM"))
        cpsum = _es.enter_context(tc.tile_pool(name="cpsum", bufs=2, space="PSUM"))
        ctxT_sb = apool.tile([P, DT4, T], BF16, name="ctxT_sb")
        wo_sb = apool.tile([P, DT4, D], BF16, name="wo_sb")
        nc.sync.dma_start(wo_sb, wo_v)
        m_strict = apool.tile([P, P], BF16, name="m_strict")
        m_incl = apool.tile([P, P], BF16, name="m_incl")
        m_diag = apool.tile([P, P], BF16, name="m_diag")
        nc.sync.dma_start(m_strict, msk[0])
        nc.sync.dma_start(m_incl, msk[1])
        nc.sync.dma_start(m_diag, msk[2])

        for hp in range(HG // 2):  # head pair: heads 2hp (rows 0:64), 2hp+1
            c = hp  # qT/kT column-tile index holding this pair
            for half in range(2):
                mask = m_strict if half == 0 else m_incl
                # ctx tile layout: row 0 = softmax denominator (via the M=1
                # ones matmul, col strip 0), rows 64..127 = the 64 v-channels
                # (col strip 64). Disjoint col strips -> the two matmuls run
                # concurrently in the PE array; same PSUM banks.
                ctxs = [
                    cpsum.tile([P, L], F32, tag="ctx", name=f"ctx{hp}_{half}_{u}")
                    for u in range(2)
                ]
                for j in range(NT):
                    span0, span1 = P * j, L
                    n = span1 - span0
                    sc = [None, None]
                    at = [None, None]
                    for u in range(2):
                        sc[u] = spsum.tile(
                            [P, 1024], F32, tag="sc", name=f"sc{hp}_{half}_{j}_{u}"
                        )
                    # row-tiled score matmuls: head u at PE rows 64u..64u+63
                    for r0, r1 in _chunks512(0, n):
                        for u in range(2):
                            p0 = HD * u
                            kv = kT_sb[p0 : p0 + HD, c, L + span0 : L + span0 + P]
                            nc.tensor.matmul(
                                sc[u][:, r0:r1],
                                kv,
                                qT_sb[
                                    p0 : p0 + HD,
                                    c,
                                    L * half + span0 + r0 : L * half + span0 + r1,
                                ],
                                start=True,
                                stop=True,
                                tile_position=(p0, 0),
                            )
                    for u in range(2):
                        at[u] = atpool.tile(
                            [P, 1024], BF16, tag="at", name=f"at{hp}_{half}_{j}_{u}"
                        )
                        nc.scalar.activation(at[u][:, :n], sc[u][:, :n], Act.Exp)
                        nc.vector.tensor_mul(at[u][:, :P], at[u][:, :P], mask)
                        if DBG and hp == 0 and half == 1 and j == 0 and u == 0:
                            nc.sync.dma_start(views["dbg"]["dbg_at"], at[u])
                    # ctx + denominator accumulation (per 512 PSUM bank)
                    for u in range(2):
                        h = 2 * hp + u
                        vj = v_sb[:, NT + j, 2 * HD * h : 2 * HD * (h + 1)]
                        for a0, a1 in _chunks512(span0, L):
                            last = half == 1 and (
                                (a1 <= 512 and j == 3) or (a0 >= 512 and j == NT - 1)
                            )
                            nc.tensor.matmul(
                                ctxs[u][:, a0:a1],
                                vj,
                                at[u][:, a0 - span0 : a1 - span0],
                                start=(j == 0),
                                stop=last,
                            )
                if half == 0:
                    # xt-xt block-diagonal tiles, batched per head
                    for u in range(2):
                        h = 2 * hp + u
                        p0 = HD * u
                        scd = spsum.tile(
                            [P, 1024], F32, tag="sc", name=f"scd{hp}_{u}"
                        )
                        for i in range(NT):
                            nc.tensor.matmul(
                                scd[:, P * i : P * (i + 1)],
                                kT_sb[p0 : p0 + HD, c, P * i : P * (i + 1)],
                                qT_sb[p0 : p0 + HD, c, P * i : P * (i + 1)],
                                start=True,
                                stop=True,
                                tile_position=(p0, 0),
                            )
                        atd = atpool.tile(
                            [P, 1024], BF16, tag="at", name=f"atd{hp}_{u}"
                        )
                        nc.scalar.activation(atd, scd, Act.Exp)
                        nc.vector.tensor_mul(
                            atd.rearrange("p (i q) -> p i q", q=P),
                            atd.rearrange("p (i q) -> p i q", q=P),
                            m_diag[:, None, :].to_broadcast((P, NT, P)),
                        )
                        if DBG and hp == 0 and u == 0:
                            nc.sync.dma_start(views["dbg"]["dbg_atd"], atd)
                        for i in range(NT):
                            nc.tensor.matmul(
                                ctxs[u][:, P * i : P * (i + 1)],
                                v_sb[:, i, 2 * HD * h : 2 * HD * (h + 1)],
                                atd[:, P * i : P * (i + 1)],
                                start=False,
                                stop=(i == 3 or i == NT - 1),
                            )
                # normalize: ctxT = ctx[64:128] * (1 / denom), denom = ctx row 0
                for u in range(2):
                    recip = tmppool.tile(
                        [1, L], F32, tag="recip", name=f"rc{hp}_{half}_{u}"
                    )
                    nc.vector.reciprocal_approx_fast(recip, ctxs[u][0:1, :])
                    # GpSimd broadcast of the recip row (physical partition 0)
                    rb = tmppool.tile([P, L], F32, tag="rb", bufs=3,
                                      name=f"rb{hp}_{half}_{u}")
                    nc.gpsimd.partition_broadcast(rb, recip, channels=P)
                    cs = tmppool.tile(
                        [P, L], BF16, tag="cs", bufs=3,
                        name=f"cs{hp}_{half}_{u}",
                    )
                    nc.vector.tensor_mul(
                        cs[HD:P, :], ctxs[u][HD:P, :], rb[HD:P, :]
                    )
                    p0 = HD * u
                    nc.sync.dma_start(
                        ctxT_sb[p0 : p0 + HD, c, L * half : L * (half + 1)],
                        cs[HD:P, :],
                    )
                    if DBG:
                        idx = hp * 4 + half * 2 + u
                        nc.sync.dma_start(views["dbg"]["dbg_recip"][idx], recip)
                        nc.sync.dma_start(views["dbg"]["dbg_rb"][idx], rb)
                        if hp == 0:
                            i2 = half * 2 + u
                            nsb = tmppool.tile(
                                [P, L], F32, tag="nsb", bufs=2,
                                name=f"nsb{half}_{u}",
                            )
                            nc.scalar.activation(
                                nsb[HD:P, :], ctxs[u][HD:P, :], Act.Identity
                            )
                            nc.sync.dma_start(
                                views["dbg"]["dbg_num"][i2], nsb[HD:P, :]
                            )
                            nc.sync.dma_start(
                                views["dbg"]["dbg_cs"][i2], cs[HD:P, :]
                            )

        if DBG:
            nc.sync.dma_start(views["dbg"]["dbg_qT"], qT_sb)
            nc.sync.dma_start(views["dbg"]["dbg_kT"], kT_sb)
            nc.sync.dma_start(views["dbg"]["dbg_v"], v_sb)
            nc.sync.dma_start(views["dbg"]["dbg_ctxT"], ctxT_sb)

        _es.close()

        # ---------------- Phase C: O-projection ----------------
        with tc.tile_pool(name="opsum", bufs=6, space="PSUM") as opsum:
            for tt in range(T // P):
                for nk in range(2):
                    ops = opsum.tile([P, 512], F32, tag="op", name=f"op{tt}_{nk}")
                    for cc in range(DT4):
                        nc.tensor.matmul(
                            ops,
                            ctxT_sb[:, cc, P * tt : P * (tt + 1)],
                            wo_sb[:, cc, 512 * nk : 512 * (nk + 1)],
                            start=(cc == 0),
                            stop=(cc == DT4 - 1),
                        )
                    osb = tmppool.tile(
                        [P, 512], F32, tag="osb", bufs=6, name=f"osb{tt}_{nk}"
                    )
                    nc.vector.tensor_copy(osb, ops)
                    nc.sync.dma_start(
                        out[P * tt : P * (tt + 1), 512 * nk : 512 * (nk + 1)], osb
                    )


def _masks():
    import ml_dtypes

    q = np.arange(P)[None, :] // BS
    k = np.arange(P)[:, None] // BS
    m = np.zeros((3, P, P), np.float32)
    m[0] = (q > k).astype(np.float32)    # strict (xt q vs x0 k, same tile)
    m[1] = (q >= k).astype(np.float32)   # incl (x0 q vs x0 k, same tile)
    m[2] = (q == k).astype(np.float32)   # diag (xt q vs xt k, same tile)
    return m.astype(ml_dtypes.bfloat16)


def kernel(x, Wq, bq, Wk, bk, Wv, bv, Wo, bo, block_size=4, **_):
    import ml_dtypes

    x = np.asarray(x, np.float32)
    Wq, bq = np.asarray(Wq, np.float32), np.asarray(bq, np.float32)
    Wk, bk = np.asarray(Wk, np.float32), np.asarray(bk, np.float32)
    Wv, bv = np.asarray(Wv, np.float32), np.asarray(bv, np.float32)
    Wo, bo = np.asarray(Wo, np.float32), np.asarray(bo, np.float32)

    if "nc" not in _CACHE:
        _CACHE["nc"] = _build()
    nc = _CACHE["nc"]

    masks = _masks()
    scale = HD ** -0.5
    in_maps = []
    for core in range(8):
        b, g = core // 2, core % 2
        cols = slice(DG * g, DG * (g + 1))
        in_maps.append(
            {
                "xT": np.ascontiguousarray(x[b].T),
                "wq": np.ascontiguousarray(Wq[:, cols]),
                "wk": np.ascontiguousarray(Wk[:, cols]),
                "wv": np.ascontiguousarray(Wv[:, cols]),
                "wo": np.ascontiguousarray(Wo[cols, :]).astype(ml_dtypes.bfloat16),
                "bqs": np.ascontiguousarray(bq[cols]) * np.float32(scale),
                "bks": np.ascontiguousarray(bk[cols]),
                "msk": masks,
            }
        )

    _CACHE["last_in_maps"] = in_maps
    last_err = None
    for _attempt in range(6):
        try:
            res = run_bass_kernel_spmd(nc, in_maps, core_ids=list(range(8)), trace=False)
            break
        except Exception as e:  # transient NRT device flakes
            last_err = e
            msg = str(e)
            if "UNRECOVERABLE" not in msg and "UNAVAILABLE" not in msg:
                raise
            import time as _time

            import jax as _jax

            _time.sleep(5 * (_attempt + 1))
            try:
                _jax.clear_backends()
            except Exception:
                pass
    else:
        raise last_err

    _CACHE["last_res"] = res
    corr = (bv @ Wo + bo).astype(np.float32)  # softmax rows sum to 1
    out = np.empty((B, T, D), np.float32)
    for b in range(B):
        out[b] = res.results[2 * b]["out"] + res.results[2 * b + 1]["out"] + corr
    return out


if __name__ == "__main__":
    rng = np.random.default_rng(0)
    inputs = {
        "x": rng.standard_normal((B, T, D)).astype(np.float32),
        "Wq": (rng.standard_normal((D, D)) / 32).astype(np.float32),
        "bq": np.zeros(D, np.float32),
        "Wk": (rng.standard_normal((D, D)) / 32).astype(np.float32),
        "bk": np.zeros(D, np.float32),
        "Wv": (rng.standard_normal((D, D)) / 32).astype(np.float32),
        "bv": np.zeros(D, np.float32),
        "Wo": (rng.standard_normal((D, D)) / 32).astype(np.float32),
        "bo": np.zeros(D, np.float32),
    }
    o = kernel(**inputs)
    print("ran", o.shape, o.dtype, float(np.abs(o).max()))


# revision 32
# speedup vs baseline: 1.7576x; 1.7576x over previous
"""BD3LM block-diffusion decoder layer on 8 trn2 NeuronCores.

Sharding: core = 2*b + g  (b = batch 0..3, g = head-group 0..1, 8 heads each).
Each core: QKV projections for its batch/head-group, sparse BD3LM attention
(only ~80 of 256 score tiles per head), O-projection against its Wo row-slice.
Host: sums the two group partials per batch and adds the (bv @ Wo + bo)
correction (softmax rows sum to 1, so the v-bias contributes exactly bv @ Wo).

v3 structure:
  - everything bf16 on the PE (1 cyc/row at any free size; f32r is 4 cyc/row
    below N=256). PSUM accumulation stays f32. End-to-end error ~5e-3.
  - x resident in SBUF; phase A emitted PAIR-major (projections for head
    pair p over the full sequence), immediately followed by attention for
    those two heads. Attention is ACT(exp)-bound, projections are PE-bound:
    the Tile scheduler overlaps pair p's exps with pair p+1's projections.
  - per head a [*,128] stationary v-block: col 0 = ones (softmax denominator
    -> ctx row 0, where reciprocal_approx_fast + gpsimd partition_broadcast
    work - both require physical partition 0), cols 64..127 = v channels
    (ctx rows 64..127, base-64-aligned for the normalize multiply).
  - scores [k_tile, q_span] into [128,1024] PSUM; ONE exp per (head, half,
    j) span and one batched exp for the 8 block-diagonal tiles (ACT per-op
    overhead ~293ns).
  - PSUM budget: proj pool 2 banks + shared score/ctx pool 6 banks.
"""

import numpy as np

import concourse.bass as bass
import concourse.mybir as mybir
import concourse.tile as tile
from concourse import bacc
from concourse.bass_utils import run_bass_kernel_spmd

F32 = mybir.dt.float32
BF16 = mybir.dt.bfloat16
Act = mybir.ActivationFunctionType

B, T, D = 4, 2048, 1024
H, HD = 16, 64
L = T // 2           # 1024, length of each of [xt | x0]
BS = 4               # block size
G = 2                # head groups (cores per batch)
DG = D // G          # 512 channels per group
HG = H // G          # 8 heads per core
P = 128
NT = L // P          # 8 key/query tiles per half
KC = D // P          # 8 contraction chunks
DT4 = DG // P        # 4 output-partition tiles for qT/kT

REPEAT = 1  # loop whole computation inside the NEFF (timing experiments only)
DBG = False

_CACHE = {}


def _chunks512(a0, a1):
    """Split [a0, a1) at multiples of 512 (PSUM bank boundaries)."""
    out = []
    while a0 < a1:
        b1 = min(a1, (a0 // 512 + 1) * 512)
        out.append((a0, b1))
        a0 = b1
    return out


def _build():
    import concourse.tile_utils as tile_utils

    tile_utils.max_sbuf_usage = 204 * 1024

    nc = bacc.Bacc("TRN2", target_bir_lowering=False, debug=False, num_devices=8)

    xT = nc.dram_tensor("xT", [D, T], BF16, kind="ExternalInput").ap()
    wq = nc.dram_tensor("wq", [D, DG], BF16, kind="ExternalInput").ap()
    wk = nc.dram_tensor("wk", [D, DG], BF16, kind="ExternalInput").ap()
    wv = nc.dram_tensor("wv", [D, DG], BF16, kind="ExternalInput").ap()
    wo = nc.dram_tensor("wo", [DG, D], BF16, kind="ExternalInput").ap()
    bqs = nc.dram_tensor("bqs", [DG], F32, kind="ExternalInput").ap()
    bks = nc.dram_tensor("bks", [DG], F32, kind="ExternalInput").ap()
    msk = nc.dram_tensor("msk", [3, P, P], BF16, kind="ExternalInput").ap()
    out = nc.dram_tensor("out", [T, D], F32, kind="ExternalOutput").ap()

    dbg = {}
    if DBG:
        for nm, shp, dt in (
            ("dbg_qT", [P, DT4, T], BF16),
            ("dbg_kT", [P, DT4, T], BF16),
            ("dbg_v", [P, T // P, HG * 2 * HD], BF16),
            ("dbg_ctxT", [P, DT4, T], BF16),
        ):
            dbg[nm] = nc.dram_tensor(nm, shp, dt, kind="ExternalOutput").ap()

    views = dict(
        dbg=dbg,
        xT_v=xT.rearrange("(kc p) t -> p kc t", p=P),    # [128, 8, 2048]
        wq_v=wq.rearrange("(kc p) m -> p kc m", p=P),    # [128, 8, 512]
        wk_v=wk.rearrange("(kc p) m -> p kc m", p=P),
        wv_v=wv.rearrange("(kc p) m -> p kc m", p=P),
        wo_v=wo.rearrange("(cc p) n -> p cc n", p=P),    # [128, 4, 1024]
        msk=msk,
        out=out,
    )

    with tile.TileContext(nc) as tc:
        with tc.tile_pool(name="persist", bufs=1) as pers:
            st = dict(
                x_sb=pers.tile([P, KC, T], BF16, name="x_sb"),
                wq_sb=pers.tile([P, KC, DG], BF16, name="wq_sb"),
                wk_sb=pers.tile([P, KC, DG], BF16, name="wk_sb"),
                wv_sb=pers.tile([P, KC, DG], BF16, name="wv_sb"),
                wo_sb=pers.tile([P, DT4, D], BF16, name="wo_sb"),
                qT_sb=pers.tile([P, DT4, T], BF16, name="qT_sb"),
                kT_sb=pers.tile([P, DT4, T], BF16, name="kT_sb"),
                v_sb=pers.tile([P, T // P, HG * 2 * HD], BF16, name="v_sb"),
                ctxT_sb=pers.tile([P, DT4, T], BF16, name="ctxT_sb"),
                bq_sb=pers.tile([P, DT4], F32, name="bq_sb"),
                bk_sb=pers.tile([P, DT4], F32, name="bk_sb"),
                m_strict=pers.tile([P, P], BF16, name="m_strict"),
                m_incl=pers.tile([P, P], BF16, name="m_incl"),
                m_diag=pers.tile([P, P], BF16, name="m_diag"),
            )
            nc.sync.dma_start(st["bq_sb"], bqs.rearrange("(c p) -> p c", p=P))
            nc.sync.dma_start(st["bk_sb"], bks.rearrange("(c p) -> p c", p=P))
            nc.sync.dma_start(st["m_strict"], msk[0])
            nc.sync.dma_start(st["m_incl"], msk[1])
            nc.sync.dma_start(st["m_diag"], msk[2])
            nc.vector.memset(st["v_sb"], 0.0)
            ones_v = st["v_sb"].rearrange("p t (h c) -> p (t h) c", c=2 * HD)[
                :, :, 0:1
            ]
            nc.vector.memset(ones_v, 1.0)

            for _rep in range(REPEAT):
                _phases(nc, tc, st, views)

    nc.compile()
    return nc


def _load(nc, st, views):
    """Stream weights + x into SBUF across both HWDGE engines (SP + ACT),
    (kc, slab)-split so the first matmuls unblock after ~1MB of traffic."""
    nc.sync.dma_start(
        st["wq_sb"][:, :, 0:P], views["wq_v"][:, :, 0:P]
    )
    nc.scalar.dma_start(
        st["wk_sb"][:, :, 0:P], views["wk_v"][:, :, 0:P]
    )
    for s in range(T // 512):
        for kc in range(KC):
            eng = nc.sync if (kc % 2 == 0) else nc.scalar
            eng.dma_start(
                st["x_sb"][:, kc, 512 * s : 512 * (s + 1)],
                views["xT_v"][:, kc, 512 * s : 512 * (s + 1)],
            )
    for d4 in range(1, DT4):
        nc.sync.dma_start(
            st["wq_sb"][:, :, P * d4 : P * (d4 + 1)],
            views["wq_v"][:, :, P * d4 : P * (d4 + 1)],
        )
        nc.scalar.dma_start(
            st["wk_sb"][:, :, P * d4 : P * (d4 + 1)],
            views["wk_v"][:, :, P * d4 : P * (d4 + 1)],
        )
    nc.sync.dma_start(st["wv_sb"], views["wv_v"])
    nc.scalar.dma_start(st["wo_sb"], views["wo_v"])


def _proj_pair(nc, st, pp, p):
    """QKV projections for head pair p (qT/kT column-tile p, v channels of
    heads 2p, 2p+1)."""
    x_sb, v_sb = st["x_sb"], st["v_sb"]
    for w_sb, b_key, dst, scale in (
        (st["wq_sb"], "bq_sb", st["qT_sb"], HD ** -0.5),
        (st["wk_sb"], "bk_sb", st["kT_sb"], 1.0),
    ):
        for s in range(T // 512):
            ps = pp.tile([P, 512], F32, tag="pp", name=f"pp{p}_{s}")
            for kc in range(KC):
                nc.tensor.matmul(
                    ps,
                    w_sb[:, kc, P * p : P * (p + 1)],
                    x_sb[:, kc, 512 * s : 512 * (s + 1)],
                    start=(kc == 0),
                    stop=(kc == KC - 1),
                )
            nc.scalar.activation(
                dst[:, p, 512 * s : 512 * (s + 1)],
                ps,
                Act.Identity,
                bias=st[b_key][:, p : p + 1],
                scale=scale,
            )
    for tt in range(T // P):
        ps = pp.tile([P, 512], F32, tag="pp", name=f"ppv{p}_{tt}")
        for kc in range(KC):
            nc.tensor.matmul(
                ps[:, :P],
                x_sb[:, kc, P * tt : P * (tt + 1)],
                st["wv_sb"][:, kc, P * p : P * (p + 1)],
                start=(kc == 0),
                stop=(kc == KC - 1),
            )
        nc.vector.tensor_copy(
            v_sb[:, tt].rearrange("p (h c) -> p h c", c=2 * HD)[
                :, 2 * p : 2 * p + 2, HD : 2 * HD
            ],
            ps[:, :P].rearrange("p (h c) -> p h c", c=HD),
        )


def _attn_head(nc, st, bpool, atpool, tmppool, h):
    """Sparse BD3LM attention + normalize for one head."""
    qT_sb, kT_sb, v_sb = st["qT_sb"], st["kT_sb"], st["v_sb"]
    c, p0 = h // 2, HD * (h % 2)
    qh = qT_sb[p0 : p0 + HD, c, :]
    kh = kT_sb[p0 : p0 + HD, c, :]
    for half in range(2):
        mask = st["m_strict"] if half == 0 else st["m_incl"]
        ctx = bpool.tile([P, L], F32, tag="ps2", name=f"ctx{h}_{half}")
        for j in range(NT):
            span0 = P * j
            n = L - span0
            sc = bpool.tile([P, 1024], F32, tag="ps2", name=f"sc{h}_{half}_{j}")
            kv = kh[:, L + span0 : L + span0 + P]
            for r0, r1 in _chunks512(0, n):
                nc.tensor.matmul(
                    sc[:, r0:r1],
                    kv,
                    qh[:, L * half + span0 + r0 : L * half + span0 + r1],
                    start=True,
                    stop=True,
                    tile_position=(p0, 0),
                )
            at = atpool.tile([P, 1024], BF16, tag="at", name=f"at{h}_{half}_{j}")
            nc.scalar.activation(at[:, :n], sc[:, :n], Act.Exp)
            nc.vector.tensor_mul(at[:, :P], at[:, :P], mask)
            vj = v_sb[:, NT + j, 2 * HD * h : 2 * HD * (h + 1)]
            for a0, a1 in _chunks512(span0, L):
                last = half == 1 and (
                    (a1 <= 512 and j == 3) or (a0 >= 512 and j == NT - 1)
                )
                nc.tensor.matmul(
                    ctx[:, a0:a1],
                    vj,
                    at[:, a0 - span0 : a1 - span0],
                    start=(j == 0),
                    stop=last,
                )
        if half == 0:
            # xt-xt block-diagonal tiles, batched exp + mask
            scd = bpool.tile([P, 1024], F32, tag="ps2", name=f"scd{h}")
            for i in range(NT):
                nc.tensor.matmul(
                    scd[:, P * i : P * (i + 1)],
                    kh[:, P * i : P * (i + 1)],
                    qh[:, P * i : P * (i + 1)],
                    start=True,
                    stop=True,
                    tile_position=(p0, 0),
                )
            atd = atpool.tile([P, 1024], BF16, tag="at", name=f"atd{h}")
            nc.scalar.activation(atd, scd, Act.Exp)
            nc.vector.tensor_mul(
                atd.rearrange("p (i q) -> p i q", q=P),
                atd.rearrange("p (i q) -> p i q", q=P),
                st["m_diag"][:, None, :].to_broadcast((P, NT, P)),
            )
            for i in range(NT):
                nc.tensor.matmul(
                    ctx[:, P * i : P * (i + 1)],
                    v_sb[:, i, 2 * HD * h : 2 * HD * (h + 1)],
                    atd[:, P * i : P * (i + 1)],
                    start=False,
                    stop=(i == 3 or i == NT - 1),
                )
        # normalize: ctxT = ctx[64:128] * (1 / denom), denom = ctx row 0
        recip = tmppool.tile([1, L], F32, tag="recip", name=f"rc{h}_{half}")
        nc.vector.reciprocal_approx_fast(recip, ctx[0:1, :])
        rb = tmppool.tile([P, L], F32, tag="rb", bufs=3, name=f"rb{h}_{half}")
        nc.gpsimd.partition_broadcast(rb, recip, channels=P)
        cs = tmppool.tile([P, L], BF16, tag="cs", bufs=3, name=f"cs{h}_{half}")
        nc.vector.tensor_mul(cs[HD:P, :], ctx[HD:P, :], rb[HD:P, :])
        nc.sync.dma_start(
            st["ctxT_sb"][p0 : p0 + HD, c, L * half : L * (half + 1)],
            cs[HD:P, :],
        )


def _phases(nc, tc, st, views):
    from contextlib import ExitStack as _ES

    _load(nc, st, views)

    with tc.tile_pool(name="tmppool", bufs=2) as tmppool:
        _es = _ES()
        atpool = _es.enter_context(tc.tile_pool(name="atpool", bufs=8))
        pp = _es.enter_context(tc.tile_pool(name="pp", bufs=2, space="PSUM"))
        bpool = _es.enter_context(tc.tile_pool(name="bpool", bufs=3, space="PSUM"))

        for p in range(DT4):
            _proj_pair(nc, st, pp, p)
            _attn_head(nc, st, bpool, atpool, tmppool, 2 * p)
            _attn_head(nc, st, bpool, atpool, tmppool, 2 * p + 1)

        if DBG:
            nc.sync.dma_start(views["dbg"]["dbg_qT"], st["qT_sb"])
            nc.sync.dma_start(views["dbg"]["dbg_kT"], st["kT_sb"])
            nc.sync.dma_start(views["dbg"]["dbg_v"], st["v_sb"])
            nc.sync.dma_start(views["dbg"]["dbg_ctxT"], st["ctxT_sb"])

        _es.close()

        # ---------------- Phase C: O-projection ----------------
        with tc.tile_pool(name="opsum", bufs=6, space="PSUM") as opsum:
            for tt in range(T // P):
                for nk in range(2):
                    ops = opsum.tile([P, 512], F32, tag="op", name=f"op{tt}_{nk}")
                    for cc in range(DT4):
                        nc.tensor.matmul(
                            ops,
                            st["ctxT_sb"][:, cc, P * tt : P * (tt + 1)],
                            st["wo_sb"][:, cc, 512 * nk : 512 * (nk + 1)],
                            start=(cc == 0),
                            stop=(cc == DT4 - 1),
                        )
                    osb = tmppool.tile(
                        [P, 512], F32, tag="osb", bufs=6, name=f"osb{tt}_{nk}"
                    )
                    nc.vector.tensor_copy(osb, ops)
                    nc.sync.dma_start(
                        views["out"][
                            P * tt : P * (tt + 1), 512 * nk : 512 * (nk + 1)
                        ],
                        osb,
                    )


def _masks():
    import ml_dtypes

    q = np.arange(P)[None, :] // BS
    k = np.arange(P)[:, None] // BS
    m = np.zeros((3, P, P), np.float32)
    m[0] = (q > k).astype(np.float32)    # strict (xt q vs x0 k, same tile)
    m[1] = (q >= k).astype(np.float32)   # incl (x0 q vs x0 k, same tile)
    m[2] = (q == k).astype(np.float32)   # diag (xt q vs xt k, same tile)
    return m.astype(ml_dtypes.bfloat16)


def kernel(x, Wq, bq, Wk, bk, Wv, bv, Wo, bo, block_size=4, **_):
    import ml_dtypes

    BF = ml_dtypes.bfloat16
    x = np.asarray(x, np.float32)
    Wq, bq = np.asarray(Wq, np.float32), np.asarray(bq, np.float32)
    Wk, bk = np.asarray(Wk, np.float32), np.asarray(bk, np.float32)
    Wv, bv = np.asarray(Wv, np.float32), np.asarray(bv, np.float32)
    Wo, bo = np.asarray(Wo, np.float32), np.asarray(bo, np.float32)

    if "nc" not in _CACHE:
        _CACHE["nc"] = _build()
    nc = _CACHE["nc"]

    masks = _masks()
    scale = HD ** -0.5
    in_maps = []
    for core in range(8):
        b, g = core // 2, core % 2
        cols = slice(DG * g, DG * (g + 1))
        in_maps.append(
            {
                "xT": np.ascontiguousarray(x[b].T).astype(BF),
                "wq": np.ascontiguousarray(Wq[:, cols]).astype(BF),
                "wk": np.ascontiguousarray(Wk[:, cols]).astype(BF),
                "wv": np.ascontiguousarray(Wv[:, cols]).astype(BF),
                "wo": np.ascontiguousarray(Wo[cols, :]).astype(BF),
                "bqs": np.ascontiguousarray(bq[cols]) * np.float32(scale),
                "bks": np.ascontiguousarray(bk[cols]),
                "msk": masks,
            }
        )

    _CACHE["last_in_maps"] = in_maps
    last_err = None
    for _attempt in range(6):
        try:
            res = run_bass_kernel_spmd(nc, in_maps, core_ids=list(range(8)), trace=False)
            break
        except Exception as e:  # transient NRT device flakes
            last_err = e
            msg = str(e)
            if "UNRECOVERABLE" not in msg and "UNAVAILABLE" not in msg:
                raise
            import time as _time

            import jax as _jax

            _time.sleep(5 * (_attempt + 1))
            try:
                _jax.clear_backends()
            except Exception:
                pass
    else:
        raise last_err

    _CACHE["last_res"] = res
    corr = (bv @ Wo + bo).astype(np.float32)  # softmax rows sum to 1
    out = np.empty((B, T, D), np.float32)
    for b in range(B):
        out[b] = res.results[2 * b]["out"] + res.results[2 * b + 1]["out"] + corr
    return out


if __name__ == "__main__":
    rng = np.random.default_rng(0)
    inputs = {
        "x": rng.standard_normal((B, T, D)).astype(np.float32),
        "Wq": (rng.standard_normal((D, D)) / 32).astype(np.float32),
        "bq": np.zeros(D, np.float32),
        "Wk": (rng.standard_normal((D, D)) / 32).astype(np.float32),
        "bk": np.zeros(D, np.float32),
        "Wv": (rng.standard_normal((D, D)) / 32).astype(np.float32),
        "bv": np.zeros(D, np.float32),
        "Wo": (rng.standard_normal((D, D)) / 32).astype(np.float32),
        "bo": np.zeros(D, np.float32),
    }
    o = kernel(**inputs)
    print("ran", o.shape, o.dtype, float(np.abs(o).max()))
